# revision 13
# baseline (speedup 1.0000x reference)
"""Trainium2 Bass kernel for nn_MoEConnectionProcessor.

Self-contained: stages/shards the full inputs on host (numpy), runs an SPMD
Bass/Tile kernel on 8 NeuronCores, gathers the full output.

Reference math (per cell, K=26 neighbors, D=32):
  masks by tier (0=local,1=functional,2=distant); masked neighbor means;
  local expert  = tanh([cs, loc_mean] @ W_local + b_local)
  func expert   = (1-z)*cs + z*tanh(agg),  z = sigmoid([cs, agg] @ W_upd + b_upd)
                  agg = masked_mean_k tanh(nb @ W_msg + b_msg)
  dist expert   = 3-step Euler: x += (1/3) tanh([x, agg_d] @ W_cnf + b_cnf)
  gates         = softmax([cs, mean_nb] @ W_g1 + b_g1 -> relu -> @ W_g2 + b_g2)
  out           = sum_t gate_t * expert_t

Device strategy (per 128-cell tile, Q=4 tiles batched for the small ops):
  - nb staged twice from host: T layout [(g,d), (c,k)] PRE-MASKED by the
    tier-1 mask (so tanh gives exact zeros for non-functional neighbors and
    the functional aggregate is a plain k-reduce), and natural [c, (d,k)]
    raw with k innermost (so the masked multiplies run in DVE 2x mode with
    the per-(cell,k) weights broadcast along d as an outer dim).
  - tier-0/tier-2 means: one fused 2x multiply by pre-divided weights
    (m_t/cnt_t, fp16) + one fused 1x k-reduce.
  - S0 (gating mean): PE identity-accumulation into PSUM (26 matmuls).
  - sigmoid via tanh identity, relu/exp/copy on ACT: every activation is
    served by the "exp_and_others" table -> zero ACT table reloads.
  - experts/gating/combine all in T layout on [128, 4*32] batched operands;
    per-cell gates/scales replicated across partitions with tiny PE matmuls;
    output staged in T layout, un-transposed on host.
"""

import numpy as np
import ml_dtypes
from contextlib import ExitStack

import concourse.bass as bass
import concourse.bacc as bacc
import concourse.tile as tile
import concourse.mybir as mybir

B, K, D, NH = 262144, 26, 32, 32
N_CORES = 8
BS = B // N_CORES   # 32768 cells per core
CT = 128            # cells per tile
QT = 8              # tiles per batch-group
N_STEPS = 3
DT_STEP = 1.0 / N_STEPS

dt = mybir.dt
bf16 = ml_dtypes.bfloat16
f16 = np.float16
AF = mybir.ActivationFunctionType
ALU = mybir.AluOpType
AXX = mybir.AxisListType

FR = K * D  # 832
TW = 2 * FR + 2 * K  # 1716: [nbt 832 | nbn 832 | aux 52] packed per tile

# weight-constant dram tensor [128, WC_COLS] bf16 layout
_WSLOTS = ["W4msg", "Wl_t", "Wl_b", "Wu_t", "Wu_b", "Wc_t", "Wc_b",
           "Wg1_t", "Wg1_b", "I128", "REP4", "REPe0", "REPe1", "REPe2"]
_WEXTRA = 24  # WG2K [128,12] + SDEN [12,12 in a 12-col slot]
WC_COLS = 128 * len(_WSLOTS) + _WEXTRA
BC_COLS = 8


def _wslot(name):
    return 128 * _WSLOTS.index(name)


def build_program(bs=BS, ct=CT):
    nt = bs // ct
    nq = nt // QT
    nc = bacc.Bacc("TRN2", target_bir_lowering=False, debug=False,
                   num_devices=N_CORES)

    a_nbig = nc.dram_tensor("nbig", [128, nt * TW], dt.bfloat16, kind="ExternalInput").ap()
    a_cstm = nc.dram_tensor("cstm", [128, nt * 64], dt.bfloat16, kind="ExternalInput").ap()
    a_wc = nc.dram_tensor("wc", [128, WC_COLS], dt.bfloat16, kind="ExternalInput").ap()
    a_bc = nc.dram_tensor("bc", [128, BC_COLS], dt.float32, kind="ExternalInput").ap()
    a_out = nc.dram_tensor("outt", [128, nt * D], dt.float32, kind="ExternalOutput").ap()

    with tile.TileContext(nc) as tc:
        _body(tc, a_nbig, a_cstm, a_wc, a_bc, a_out, bs, ct, nt, nq)
    nc.compile()
    return nc


def _body(tc, a_nbig, a_cstm, a_wc, a_bc, a_out, bs, ct, nt, nq):
    nc = tc.nc

    with ExitStack() as ctx:
        ctx.enter_context(nc.allow_low_precision("reduce output downcast; fp32 internal accum"))
        cpool = ctx.enter_context(tc.tile_pool(name="const", bufs=1))
        pin = ctx.enter_context(tc.tile_pool(name="in", bufs=3))
        pmid = ctx.enter_context(tc.tile_pool(name="mid", bufs=3))
        pq = ctx.enter_context(tc.tile_pool(name="q", bufs=3))
        psm = ctx.enter_context(tc.tile_pool(name="psm", bufs=1, space="PSUM"))
        psq = ctx.enter_context(tc.tile_pool(name="psq", bufs=2, space="PSUM"))
        psb = ctx.enter_context(tc.tile_pool(name="psb", bufs=1, space="PSUM"))

        wc = cpool.tile([128, WC_COLS], dt.bfloat16, tag="wc")
        nc.sync.dma_start(wc[:], a_wc)
        bc = cpool.tile([128, BC_COLS], dt.float32, tag="bc")
        nc.sync.dma_start(bc[:], a_bc)

        def W(name):
            return wc[:, _wslot(name): _wslot(name) + 128]

        wg2k = wc[:, 128 * len(_WSLOTS): 128 * len(_WSLOTS) + 12]
        sden = wc[0:12, 128 * len(_WSLOTS) + 12: 128 * len(_WSLOTS) + 24]
        b_msg4 = bc[:, 0:1]
        b_loc4 = bc[:, 1:2]
        hb_upd4 = bc[:, 2:3]   # 0.5 * b_upd (for the tanh-sigmoid identity)
        b_cnf4 = bc[:, 3:4]
        b_g14 = bc[:, 4:5]
        bg2c = bc[0:12, 5:6]   # b_g2 at (g,e) partitions

        def phase_a(q):
            ML = pq.tile([128, QT * 2 * D], dt.bfloat16, tag="ML")   # [c,(t,s,d)]
            SGT = pq.tile([128, QT * D], dt.bfloat16, tag="SGT")     # [(g,j),(t,c)]
            ps_s0 = psq.tile([128, QT * D], dt.float32, tag="ps_s0")  # [c,(t,d)]
            cm = pin.tile([128, QT * 64], dt.bfloat16, tag="cm")
            nc.sync.dma_start(cm[:], a_cstm[:, q * QT * 64:(q + 1) * QT * 64])

            mlv = ML[:].rearrange("p (t s d) -> p t s d", t=QT, s=2)
            pend = None   # (s, padd, spad) awaiting reduction, one tile delayed

            def tails(item):
                s, padd, spad = item
                pav = padd[:].rearrange("p (s d k) -> p s d k", s=2, d=D)
                nc.vector.tensor_reduce(out=mlv[:, s], in_=pav,
                                        axis=AXX.X, op=ALU.add)
                spv = spad[:].rearrange("p (c k) -> p c k", k=13)
                nc.vector.tensor_reduce(out=SGT[:, s * D:(s + 1) * D],
                                        in_=spv, axis=AXX.X, op=ALU.add)

            for s in range(QT):
                t = q * QT + s

                big = pin.tile([128, TW], dt.bfloat16, tag="big")
                nc.sync.dma_start(big[:], a_nbig[:, t * TW:(t + 1) * TW])
                nbt = big[:, 0:FR]
                nbn = big[:, FR:2 * FR]
                aux = big[:, 2 * FR:TW].bitcast(dt.float16)

                nbn3 = nbn.rearrange("p (d k) -> p d k", d=D)

                # tier-0 product on DVE, tier-2 product on GpSimd
                prod = pmid.tile([128, 2 * FR], dt.bfloat16, tag="prod")
                pview = prod[:].rearrange("p (s d k) -> p s d k", s=2, d=D)
                aview = aux.rearrange("p (s k) -> p s k", s=2)
                nc.vector.tensor_tensor(
                    out=pview[:, 0],
                    in0=nbn3,
                    in1=aview[:, 0].unsqueeze(1).to_broadcast((128, D, K)),
                    op=ALU.mult)
                nc.gpsimd.tensor_tensor(
                    out=pview[:, 1],
                    in0=nbn3,
                    in1=aview[:, 1].unsqueeze(1).to_broadcast((128, D, K)),
                    op=ALU.mult)

                # halve k by pair-adds: tier-0 half on DVE, tier-2 on GpSimd
                padd = pmid.tile([128, 2 * D * 13], dt.bfloat16, tag="padd")
                pav = padd[:].rearrange("p (s d k) -> p s d k", s=2, d=D)
                nc.vector.tensor_tensor(out=pav[:, 0], in0=pview[:, 0, :, 0:13],
                                        in1=pview[:, 0, :, 13:26], op=ALU.add)
                nc.gpsimd.tensor_tensor(out=pav[:, 1], in0=pview[:, 1, :, 0:13],
                                        in1=pview[:, 1, :, 13:26], op=ALU.add)

                # messages: premsgs = kron4(W_msg) @ nbt  (nbt pre-masked m1)
                ps_m0 = psm.tile([128, 416], dt.float32, tag="psm0")
                ps_m1 = psm.tile([128, 416], dt.float32, tag="psm1")
                nc.tensor.matmul(ps_m0[:], W("W4msg"), nbt[:, 0:416],
                                 start=True, stop=True)
                nc.tensor.matmul(ps_m1[:], W("W4msg"), nbt[:, 416:832],
                                 start=True, stop=True)
                mt = pmid.tile([128, FR], dt.bfloat16, tag="msgsT")
                nc.scalar.activation(mt[:, 0:416], ps_m0[:], AF.Tanh,
                                     bias=b_msg4, scale=1.0)
                nc.scalar.activation(mt[:, 416:832], ps_m1[:], AF.Tanh,
                                     bias=b_msg4, scale=1.0)

                # functional aggregate pair-add on GpSimd
                mt3 = mt[:].rearrange("p (c k) -> p c k", k=K)
                spad = pmid.tile([128, D * 13], dt.bfloat16, tag="spad")
                spv = spad[:].rearrange("p (c k) -> p c k", k=13)
                nc.gpsimd.tensor_tensor(out=spv, in0=mt3[:, :, 0:13],
                                        in1=mt3[:, :, 13:26], op=ALU.add)

                # S0 = sum_k nb  (PE identity accumulation)
                for k in range(K):
                    nc.tensor.matmul(ps_s0[:, s * D:(s + 1) * D], W("I128"),
                                     nbn3[:, :, k],
                                     start=(k == 0), stop=(k == K - 1))

                # one-tile-delayed reduces keep both queues unblocked
                if pend is not None:
                    tails(pend)
                pend = (s, padd, spad)
            tails(pend)

            return dict(ML=ML, SGT=SGT, ps_s0=ps_s0, cm=cm, q=q)

        def phase_b(st):
            ML, SGT, ps_s0, cm, q = st["ML"], st["SGT"], st["ps_s0"], st["cm"], st["q"]

            cmv = cm[:].rearrange("p (t x) -> p t x", x=64)
            cstqt = pq.tile([128, QT * D], dt.bfloat16, tag="cstqt")
            nc.vector.tensor_copy(cstqt[:], cmv[:, :, 0:D])
            cstq = cstqt[:]
            sclq4 = cm[0:4, :].rearrange("p (t x) -> p t x", x=64)[:, :, D:2 * D]

            # T-transpose of the tier means (32x32 block transpose)
            MLT = pq.tile([128, QT * 2 * D], dt.bfloat16, tag="MLT")
            nc.vector.transpose(MLT[:], ML[:])
            mltv = MLT[:].rearrange("p (t s c) -> p t s c", t=QT, s=2)
            mlT = mltv[:, :, 0]   # [(g,d), (t,c)] tier-0 means
            mdT = mltv[:, :, 1]   # tier-2 means

            # S0 -> bf16 -> T layout (gating mean; 1/K folded into Wg1_b)
            s0b = pq.tile([128, QT * D], dt.bfloat16, tag="s0b")
            nc.scalar.copy(s0b[:], ps_s0[:])
            mnT = pq.tile([128, QT * D], dt.bfloat16, tag="mnT")
            nc.vector.transpose(mnT[:], s0b[:])

            # shared PSUM bank tiles for all phase-B matmul outputs
            QD = QT * D
            PB1a = psb.tile([128, 2 * QD], dt.float32, tag="PB1a")
            PB1b = psb.tile([128, 2 * QD], dt.float32, tag="PB1b")
            PB2 = psb.tile([128, 2 * QD], dt.float32, tag="PB2")
            ps_l = PB1a[:, 0:QD]
            ps_z = PB1a[:, QD:2 * QD]
            ps_h = PB1b[:, 0:QD]
            ps_c = PB1b[:, QD:2 * QD]
            ps_x = PB2[:, 0:QD]      # reused: scl-rep, gating logits, denom
            ps_we = PB2[:, QD:2 * QD]

            # replicate 1/cnt1 across partitions; scale + tanh the aggregate
            nc.tensor.matmul(ps_x, W("REP4")[0:4, :], sclq4,
                             start=True, stop=True)
            aggTs = pq.tile([128, QT * D], dt.bfloat16, tag="aggTs")
            nc.vector.tensor_tensor(out=aggTs[:], in0=SGT[:], in1=ps_x,
                                    op=ALU.mult)
            taggT = pq.tile([128, QT * D], dt.bfloat16, tag="taggT")
            nc.scalar.activation(taggT[:], aggTs[:], AF.Tanh)

            # ---- experts (batched matmuls over [128, QT*32]) ----
            nc.tensor.matmul(ps_l, W("Wl_t"), cstq, start=True, stop=False)
            nc.tensor.matmul(ps_l, W("Wl_b"), mlT, start=False, stop=True)
            expcat = pq.tile([128, 3 * QT * D], dt.bfloat16, tag="expcat")
            localT = expcat[:, 0:QT * D]
            nc.scalar.activation(localT, ps_l, AF.Tanh, bias=b_loc4, scale=1.0)

            nc.tensor.matmul(ps_z, W("Wu_t"), cstq, start=True, stop=False)
            nc.tensor.matmul(ps_z, W("Wu_b"), aggTs[:], start=False, stop=True)
            tauT = pq.tile([128, QT * D], dt.bfloat16, tag="tauT")
            nc.scalar.activation(tauT[:], ps_z, AF.Tanh, bias=hb_upd4, scale=0.5)

            nc.tensor.matmul(ps_h, W("Wg1_t"), cstq, start=True, stop=False)
            nc.tensor.matmul(ps_h, W("Wg1_b"), mnT[:], start=False, stop=True)
            hT = pq.tile([128, QT * D], dt.bfloat16, tag="hT")
            nc.scalar.activation(hT[:], ps_h, AF.Relu, bias=b_g14, scale=1.0)

            # CNF: 3 Euler steps in T layout (x kept bf16)
            xcur = cstqt
            for st_i in range(N_STEPS):
                xm = xcur[:]
                nc.tensor.matmul(ps_c, W("Wc_t"), xm, start=True, stop=False)
                nc.tensor.matmul(ps_c, W("Wc_b"), mdT, start=False, stop=True)
                vT = pq.tile([128, QT * D], dt.bfloat16, tag=f"vT{st_i}")
                nc.scalar.activation(vT[:], ps_c, AF.Tanh, bias=b_cnf4, scale=1.0)
                if st_i < N_STEPS - 1:
                    xn = pq.tile([128, QT * D], dt.bfloat16, tag=f"xn{st_i}")
                    xo = xn[:]
                else:
                    xo = expcat[:, 2 * QT * D:3 * QT * D]
                nc.vector.scalar_tensor_tensor(out=xo, in0=vT[:], scalar=DT_STEP,
                                               in1=xm, op0=ALU.mult, op1=ALU.add)
                xcur = xn if st_i < N_STEPS - 1 else None

            # func expert: cs + (0.5 + 0.5*tau) * (tanh(agg) - cs)
            d2 = pq.tile([128, QT * D], dt.bfloat16, tag="d2")
            nc.vector.tensor_tensor(out=d2[:], in0=taggT[:], in1=cstq,
                                    op=ALU.subtract)
            f1 = pq.tile([128, QT * D], dt.bfloat16, tag="f1")
            nc.vector.scalar_tensor_tensor(out=f1[:], in0=tauT[:], scalar=1.0,
                                           in1=d2[:], op0=ALU.add, op1=ALU.mult)
            funcT = expcat[:, QT * D:2 * QT * D]
            nc.vector.scalar_tensor_tensor(out=funcT, in0=f1[:], scalar=0.5,
                                           in1=cstq, op0=ALU.mult, op1=ALU.add)

            # ---- gating: softmax over 3 experts ----
            nc.tensor.matmul(ps_x[0:12, 0:QD], wg2k, hT[:], start=True, stop=True)
            eg = pq.tile([128, QT * D], dt.bfloat16, tag="eg")
            nc.scalar.activation(eg[0:12, :], ps_x[0:12, 0:QD], AF.Exp,
                                 bias=bg2c, scale=1.0)
            nc.tensor.matmul(ps_x[0:12, 0:QD], sden, eg[0:12, :], start=True, stop=True)
            rinv = pq.tile([128, QT * D], dt.float32, tag="rinv")
            nc.vector.reciprocal_approx_fast(rinv[0:12, :], ps_x[0:12, 0:QD])
            gts = pq.tile([128, QT * D], dt.bfloat16, tag="gts")
            nc.vector.tensor_tensor(out=gts[0:12, :], in0=eg[0:12, :],
                                    in1=rinv[0:12, :], op=ALU.mult)

            # replicate gates over feature partitions: [(g,d), (t,c)] x3
            wrep = pq.tile([128, 3 * QT * D], dt.bfloat16, tag="wrep")
            for e in range(3):
                nc.tensor.matmul(ps_we, W(f"REPe{e}")[0:12, :], gts[0:12, :],
                                 start=True, stop=True)
                nc.scalar.copy(wrep[:, e * QT * D:(e + 1) * QT * D], ps_we)

            # ---- weighted combine: one wide gate-multiply, then 2 adds ----
            ac = pq.tile([128, 3 * QT * D], dt.bfloat16, tag="ac")
            nc.vector.tensor_tensor(out=ac[:], in0=expcat[:], in1=wrep[:],
                                    op=ALU.mult)
            s01 = pq.tile([128, QT * D], dt.bfloat16, tag="s01")
            nc.vector.tensor_tensor(out=s01[:], in0=ac[:, 0:QT * D],
                                    in1=ac[:, QT * D:2 * QT * D], op=ALU.add)
            outq = pq.tile([128, QT * D], dt.float32, tag="outq")
            nc.vector.tensor_tensor(out=outq[:], in0=s01[:],
                                    in1=ac[:, 2 * QT * D:3 * QT * D], op=ALU.add)

            nc.sync.dma_start(a_out[:, q * QT * D:(q + 1) * QT * D], outq[:])

        # software pipeline: phase B runs one group behind phase A
        prev = None
        for q in range(nq):
            cur = phase_a(q)
            if prev is not None:
                phase_b(prev)
            prev = cur
        phase_b(prev)


# ---------------------------------------------------------------------------
# host staging
# ---------------------------------------------------------------------------

def _kron4(w):
    return np.kron(np.eye(4, dtype=np.float32), w)


def stage_inputs(inputs, bs=BS, ct=CT):
    nt = bs // ct
    f32 = np.float32
    cs = np.asarray(inputs["current_state"], f32)
    nb = np.asarray(inputs["neighbor_states"], f32)
    tiers = np.asarray(inputs["tier_ids"], np.int32)

    W_local = np.asarray(inputs["W_local"], f32)
    W_msg = np.asarray(inputs["W_msg"], f32)
    W_upd = np.asarray(inputs["W_upd"], f32)
    W_cnf = np.asarray(inputs["W_cnf"], f32)
    W_g1 = np.asarray(inputs["W_g1"], f32)
    W_g2 = np.asarray(inputs["W_g2"], f32)
    b_msg = np.asarray(inputs["b_msg"], f32)
    b_local = np.asarray(inputs["b_local"], f32)
    b_upd = np.asarray(inputs["b_upd"], f32)
    b_cnf = np.asarray(inputs["b_cnf"], f32)
    b_g1 = np.asarray(inputs["b_g1"], f32)
    b_g2 = np.asarray(inputs["b_g2"], f32)

    # --- weight constants ---
    wcq = np.zeros((128, WC_COLS), f32)

    def put(name, m):
        wcq[:m.shape[0], _wslot(name):_wslot(name) + m.shape[1]] = m

    put("W4msg", _kron4(W_msg))
    put("Wl_t", _kron4(W_local[:D]))
    put("Wl_b", _kron4(W_local[D:]))
    put("Wu_t", _kron4(W_upd[:D]))
    put("Wu_b", _kron4(W_upd[D:]))
    put("Wc_t", _kron4(W_cnf[:D]))
    put("Wc_b", _kron4(W_cnf[D:]))
    put("Wg1_t", _kron4(W_g1[:D]))
    put("Wg1_b", _kron4(W_g1[D:] / K))
    put("I128", np.eye(128, dtype=f32))
    put("REP4", np.kron(np.eye(4, dtype=f32), np.ones((1, D), f32)))
    for e in range(3):
        sel = np.zeros((3, D), f32)
        sel[e, :] = 1.0
        put(f"REPe{e}", np.kron(np.eye(4, dtype=f32), sel))
    base = 128 * len(_WSLOTS)
    wcq[:, base:base + 12] = np.kron(np.eye(4, dtype=f32), W_g2)
    wcq[0:12, base + 12:base + 24] = np.kron(np.eye(4, dtype=f32),
                                             np.ones((3, 3), f32))
    wcq = wcq.astype(bf16)

    bcq = np.zeros((128, BC_COLS), f32)
    bcq[:, 0] = np.tile(b_msg, 4)
    bcq[:, 1] = np.tile(b_local, 4)
    bcq[:, 2] = 0.5 * np.tile(b_upd, 4)
    bcq[:, 3] = np.tile(b_cnf, 4)
    bcq[:, 4] = np.tile(b_g1, 4)
    bcq[0:12, 5] = np.tile(b_g2, 4)

    in_maps = []
    for c in range(N_CORES):
        rs = slice(c * bs, (c + 1) * bs)
        nb_c = nb[rs]
        cs_c = cs[rs]
        tr_c = tiers[rs]

        m0 = (tr_c == 0)
        m1 = (tr_c == 1)
        m2 = (tr_c == 2)
        cnt0 = np.maximum(m0.sum(-1), 1).astype(f32)
        cnt1 = np.maximum(m1.sum(-1), 1).astype(f32)
        cnt2 = np.maximum(m2.sum(-1), 1).astype(f32)

        # T layout, tier-1 premasked: nbt[(g,d), t, (c,k)]
        nbm = nb_c * m1[:, :, None].astype(f32)
        arr = nbm.reshape(nt, 4, 32, K, D).transpose(1, 4, 0, 2, 3)
        nbt = np.ascontiguousarray(arr).reshape(128, nt, FR).astype(bf16)

        # natural (d,k): nbn[cell, d*K + k]
        nbn = nb_c.transpose(0, 2, 1).reshape(nt, 128, FR).astype(bf16)

        aux = np.empty((bs, 2 * K), f16)
        aux[:, 0:K] = (m0 / cnt0[:, None]).astype(f16)
        aux[:, K:2 * K] = (m2 / cnt2[:, None]).astype(f16)

        nbig = np.empty((128, nt, TW), np.uint16)
        nbig[:, :, 0:FR] = nbt.view(np.uint16)
        nbig[:, :, FR:2 * FR] = nbn.view(np.uint16).transpose(1, 0, 2)
        nbig[:, :, 2 * FR:TW] = aux.view(np.uint16) \
            .reshape(nt, 128, 2 * K).transpose(1, 0, 2)
        nbig = nbig.reshape(128, nt * TW).view(bf16)

        cst = cs_c.reshape(nt, 4, 32, D).transpose(1, 3, 0, 2) \
            .reshape(128, nt, D).astype(bf16)
        scl = (1.0 / cnt1).reshape(nt, 4, 32).transpose(1, 0, 2) \
            .reshape(4, nt, D).astype(bf16)
        cstm = np.zeros((128, nt, 64), bf16)
        cstm[:, :, 0:D] = cst
        cstm[0:4, :, D:2 * D] = scl
        cstm = cstm.reshape(128, nt * 64)

        in_maps.append({
            "nbig": nbig, "cstm": cstm, "wc": wcq, "bc": bcq,
        })
    return in_maps


def unstage_output(outt, bs=BS, ct=CT):
    """outt [128, nt*D] T layout -> [bs, D] natural."""
    nt = bs // ct
    return np.ascontiguousarray(
        outt.reshape(4, D, nt, 32).transpose(2, 0, 3, 1).reshape(bs, D))


_PROGRAM_CACHE = {}


def kernel(**inputs):
    from concourse.bass_utils import run_bass_kernel_spmd

    key = (BS, CT)
    if key not in _PROGRAM_CACHE:
        _PROGRAM_CACHE[key] = build_program(BS, CT)
    nc = _PROGRAM_CACHE[key]

    in_maps = stage_inputs(inputs, BS, CT)
    res = run_bass_kernel_spmd(nc, in_maps, core_ids=list(range(N_CORES)))
    out = np.concatenate(
        [unstage_output(r["outt"].astype(np.float32)) for r in res.results],
        axis=0)
    return out.astype(np.float32)


# revision 14
# speedup vs baseline: 1.0765x; 1.0765x over previous
"""Trainium2 Bass kernel for nn_MoEConnectionProcessor.

Self-contained: stages/shards the full inputs on host (numpy), runs an SPMD
Bass/Tile kernel on 8 NeuronCores, gathers the full output.

Reference math (per cell, K=26 neighbors, D=32):
  masks by tier (0=local,1=functional,2=distant); masked neighbor means;
  local expert  = tanh([cs, loc_mean] @ W_local + b_local)
  func expert   = (1-z)*cs + z*tanh(agg),  z = sigmoid([cs, agg] @ W_upd + b_upd)
                  agg = masked_mean_k tanh(nb @ W_msg + b_msg)
  dist expert   = 3-step Euler: x += (1/3) tanh([x, agg_d] @ W_cnf + b_cnf)
  gates         = softmax([cs, mean_nb] @ W_g1 + b_g1 -> relu -> @ W_g2 + b_g2)
  out           = sum_t gate_t * expert_t

Device strategy (per 128-cell tile, Q=4 tiles batched for the small ops):
  - nb staged twice from host: T layout [(g,d), (c,k)] PRE-MASKED by the
    tier-1 mask (so tanh gives exact zeros for non-functional neighbors and
    the functional aggregate is a plain k-reduce), and natural [c, (d,k)]
    raw with k innermost (so the masked multiplies run in DVE 2x mode with
    the per-(cell,k) weights broadcast along d as an outer dim).
  - tier-0/tier-2 means: one fused 2x multiply by pre-divided weights
    (m_t/cnt_t, fp16) + one fused 1x k-reduce.
  - S0 (gating mean): PE identity-accumulation into PSUM (26 matmuls).
  - sigmoid via tanh identity, relu/exp/copy on ACT: every activation is
    served by the "exp_and_others" table -> zero ACT table reloads.
  - experts/gating/combine all in T layout on [128, 4*32] batched operands;
    per-cell gates/scales replicated across partitions with tiny PE matmuls;
    output staged in T layout, un-transposed on host.
"""

import numpy as np
import ml_dtypes
from contextlib import ExitStack

import concourse.bass as bass
import concourse.bacc as bacc
import concourse.tile as tile
import concourse.mybir as mybir

B, K, D, NH = 262144, 26, 32, 32
N_CORES = 8
BS = B // N_CORES   # 32768 cells per core
CT = 128            # cells per tile
QT = 8              # tiles per batch-group
N_STEPS = 3
DT_STEP = 1.0 / N_STEPS

dt = mybir.dt
bf16 = ml_dtypes.bfloat16
f16 = np.float16
AF = mybir.ActivationFunctionType
ALU = mybir.AluOpType
AXX = mybir.AxisListType

FR = K * D  # 832
TW = 2 * FR + 2 * K  # 1716: [nbt 832 | nbn 832 | aux 52] packed per tile

# weight-constant dram tensor [128, WC_COLS] bf16 layout
_WSLOTS = ["W4msg", "Wl_t", "Wl_b", "Wu_t", "Wu_b", "Wc_t", "Wc_b",
           "Wg1_t", "Wg1_b", "I128", "REP4", "REPe0", "REPe1", "REPe2"]
_WEXTRA = 24  # WG2K [128,12] + SDEN [12,12 in a 12-col slot]
WC_COLS = 128 * len(_WSLOTS) + _WEXTRA
BC_COLS = 8


def _wslot(name):
    return 128 * _WSLOTS.index(name)


def build_program(bs=BS, ct=CT):
    nt = bs // ct
    nq = nt // QT
    nc = bacc.Bacc("TRN2", target_bir_lowering=False, debug=False,
                   num_devices=N_CORES)

    a_nbig = nc.dram_tensor("nbig", [128, nt * TW], dt.bfloat16, kind="ExternalInput").ap()
    a_cstm = nc.dram_tensor("cstm", [128, nt * 64], dt.bfloat16, kind="ExternalInput").ap()
    a_wc = nc.dram_tensor("wc", [128, WC_COLS], dt.bfloat16, kind="ExternalInput").ap()
    a_bc = nc.dram_tensor("bc", [128, BC_COLS], dt.float32, kind="ExternalInput").ap()
    a_out = nc.dram_tensor("outt", [128, nt * D], dt.float32, kind="ExternalOutput").ap()

    with tile.TileContext(nc) as tc:
        _body(tc, a_nbig, a_cstm, a_wc, a_bc, a_out, bs, ct, nt, nq)
    nc.compile()
    return nc


def _body(tc, a_nbig, a_cstm, a_wc, a_bc, a_out, bs, ct, nt, nq):
    nc = tc.nc

    with ExitStack() as ctx:
        ctx.enter_context(nc.allow_low_precision("reduce output downcast; fp32 internal accum"))
        cpool = ctx.enter_context(tc.tile_pool(name="const", bufs=1))
        pin = ctx.enter_context(tc.tile_pool(name="in", bufs=4))
        pmid = ctx.enter_context(tc.tile_pool(name="mid", bufs=4))
        pq = ctx.enter_context(tc.tile_pool(name="q", bufs=3))
        psm = ctx.enter_context(tc.tile_pool(name="psm", bufs=1, space="PSUM"))
        psq = ctx.enter_context(tc.tile_pool(name="psq", bufs=2, space="PSUM"))
        psb = ctx.enter_context(tc.tile_pool(name="psb", bufs=1, space="PSUM"))

        wc = cpool.tile([128, WC_COLS], dt.bfloat16, tag="wc")
        nc.sync.dma_start(wc[:], a_wc)
        bc = cpool.tile([128, BC_COLS], dt.float32, tag="bc")
        nc.sync.dma_start(bc[:], a_bc)

        def W(name):
            return wc[:, _wslot(name): _wslot(name) + 128]

        wg2k = wc[:, 128 * len(_WSLOTS): 128 * len(_WSLOTS) + 12]
        sden = wc[0:12, 128 * len(_WSLOTS) + 12: 128 * len(_WSLOTS) + 24]
        b_msg4 = bc[:, 0:1]
        b_loc4 = bc[:, 1:2]
        hb_upd4 = bc[:, 2:3]   # 0.5 * b_upd (for the tanh-sigmoid identity)
        b_cnf4 = bc[:, 3:4]
        b_g14 = bc[:, 4:5]
        bg2c = bc[0:12, 5:6]   # b_g2 at (g,e) partitions

        def phase_a(q):
            ML = pq.tile([128, QT * 2 * D], dt.bfloat16, tag="ML")   # [c,(t,s,d)]
            SGT = pq.tile([128, QT * D], dt.bfloat16, tag="SGT")     # [(g,j),(t,c)]
            ps_s0 = psq.tile([128, QT * D], dt.float32, tag="ps_s0")  # [c,(t,d)]
            cm = pin.tile([128, QT * 64], dt.bfloat16, tag="cm")
            nc.sync.dma_start(cm[:], a_cstm[:, q * QT * 64:(q + 1) * QT * 64])

            for s in range(QT):
                t = q * QT + s

                big = pin.tile([128, TW], dt.bfloat16, tag="big")
                nc.sync.dma_start(big[:], a_nbig[:, t * TW:(t + 1) * TW])
                nbt = big[:, 0:FR]
                nbn = big[:, FR:2 * FR]
                aux = big[:, 2 * FR:TW].bitcast(dt.float16)

                nbn3 = nbn.rearrange("p (d k) -> p d k", d=D)

                # tier-0 product on DVE, tier-2 product on GpSimd
                prod = pmid.tile([128, 2 * FR], dt.bfloat16, tag="prod")
                pview = prod[:].rearrange("p (s d k) -> p s d k", s=2, d=D)
                aview = aux.rearrange("p (s k) -> p s k", s=2)
                nc.vector.tensor_tensor(
                    out=pview[:, 0],
                    in0=nbn3,
                    in1=aview[:, 0].unsqueeze(1).to_broadcast((128, D, K)),
                    op=ALU.mult)
                nc.gpsimd.tensor_tensor(
                    out=pview[:, 1],
                    in0=nbn3,
                    in1=aview[:, 1].unsqueeze(1).to_broadcast((128, D, K)),
                    op=ALU.mult)

                # halve k by one 2x pair-add, then 1x-reduce 13 -> means
                padd = pmid.tile([128, 2 * D * 13], dt.bfloat16, tag="padd")
                pav = padd[:].rearrange("p (s d k) -> p s d k", s=2, d=D)
                nc.vector.tensor_tensor(out=pav, in0=pview[:, :, :, 0:13],
                                        in1=pview[:, :, :, 13:26], op=ALU.add)
                mlv = ML[:].rearrange("p (t s d) -> p t s d", t=QT, s=2)
                nc.vector.tensor_reduce(out=mlv[:, s], in_=pav,
                                        axis=AXX.X, op=ALU.add)

                # messages: premsgs = kron4(W_msg) @ nbt  (nbt pre-masked m1)
                ps_m0 = psm.tile([128, 416], dt.float32, tag="psm0")
                ps_m1 = psm.tile([128, 416], dt.float32, tag="psm1")
                nc.tensor.matmul(ps_m0[:], W("W4msg"), nbt[:, 0:416],
                                 start=True, stop=True)
                nc.tensor.matmul(ps_m1[:], W("W4msg"), nbt[:, 416:832],
                                 start=True, stop=True)
                mt = pmid.tile([128, FR], dt.bfloat16, tag="msgsT")
                nc.scalar.activation(mt[:, 0:416], ps_m0[:], AF.Tanh,
                                     bias=b_msg4, scale=1.0)
                nc.scalar.activation(mt[:, 416:832], ps_m1[:], AF.Tanh,
                                     bias=b_msg4, scale=1.0)

                # functional aggregate: 2x pair-add + 1x k-reduce
                mt3 = mt[:].rearrange("p (c k) -> p c k", k=K)
                spad = pmid.tile([128, D * 13], dt.bfloat16, tag="spad")
                spv = spad[:].rearrange("p (c k) -> p c k", k=13)
                nc.vector.tensor_tensor(out=spv, in0=mt3[:, :, 0:13],
                                        in1=mt3[:, :, 13:26], op=ALU.add)
                nc.vector.tensor_reduce(out=SGT[:, s * D:(s + 1) * D],
                                        in_=spv, axis=AXX.X, op=ALU.add)

                # S0 = sum_k nb  (PE identity accumulation)
                for k in range(K):
                    nc.tensor.matmul(ps_s0[:, s * D:(s + 1) * D], W("I128"),
                                     nbn3[:, :, k],
                                     start=(k == 0), stop=(k == K - 1))

            return dict(ML=ML, SGT=SGT, ps_s0=ps_s0, cm=cm, q=q)

        def phase_b(st):
            ML, SGT, ps_s0, cm, q = st["ML"], st["SGT"], st["ps_s0"], st["cm"], st["q"]

            cmv = cm[:].rearrange("p (t x) -> p t x", x=64)
            cstqt = pq.tile([128, QT * D], dt.bfloat16, tag="cstqt")
            nc.vector.tensor_copy(cstqt[:], cmv[:, :, 0:D])
            cstq = cstqt[:]
            sclq4 = cm[0:4, :].rearrange("p (t x) -> p t x", x=64)[:, :, D:2 * D]

            # T-transpose of the tier means (32x32 block transpose)
            MLT = pq.tile([128, QT * 2 * D], dt.bfloat16, tag="MLT")
            nc.vector.transpose(MLT[:], ML[:])
            mltv = MLT[:].rearrange("p (t s c) -> p t s c", t=QT, s=2)
            mlT = mltv[:, :, 0]   # [(g,d), (t,c)] tier-0 means
            mdT = mltv[:, :, 1]   # tier-2 means

            # S0 -> bf16 -> T layout (gating mean; 1/K folded into Wg1_b)
            s0b = pq.tile([128, QT * D], dt.bfloat16, tag="s0b")
            nc.scalar.copy(s0b[:], ps_s0[:])
            mnT = pq.tile([128, QT * D], dt.bfloat16, tag="mnT")
            nc.vector.transpose(mnT[:], s0b[:])

            # shared PSUM bank tiles for all phase-B matmul outputs
            QD = QT * D
            PB1a = psb.tile([128, 2 * QD], dt.float32, tag="PB1a")
            PB1b = psb.tile([128, 2 * QD], dt.float32, tag="PB1b")
            PB2 = psb.tile([128, 2 * QD], dt.float32, tag="PB2")
            ps_l = PB1a[:, 0:QD]
            ps_z = PB1a[:, QD:2 * QD]
            ps_h = PB1b[:, 0:QD]
            ps_c = PB1b[:, QD:2 * QD]
            ps_x = PB2[:, 0:QD]      # reused: scl-rep, gating logits, denom
            ps_we = PB2[:, QD:2 * QD]

            # replicate 1/cnt1 across partitions; scale + tanh the aggregate
            nc.tensor.matmul(ps_x, W("REP4")[0:4, :], sclq4,
                             start=True, stop=True)
            aggTs = pq.tile([128, QT * D], dt.bfloat16, tag="aggTs")
            nc.vector.tensor_tensor(out=aggTs[:], in0=SGT[:], in1=ps_x,
                                    op=ALU.mult)
            taggT = pq.tile([128, QT * D], dt.bfloat16, tag="taggT")
            nc.scalar.activation(taggT[:], aggTs[:], AF.Tanh)

            # ---- experts (batched matmuls over [128, QT*32]) ----
            nc.tensor.matmul(ps_l, W("Wl_t"), cstq, start=True, stop=False)
            nc.tensor.matmul(ps_l, W("Wl_b"), mlT, start=False, stop=True)
            expcat = pq.tile([128, 3 * QT * D], dt.bfloat16, tag="expcat")
            localT = expcat[:, 0:QT * D]
            nc.scalar.activation(localT, ps_l, AF.Tanh, bias=b_loc4, scale=1.0)

            nc.tensor.matmul(ps_z, W("Wu_t"), cstq, start=True, stop=False)
            nc.tensor.matmul(ps_z, W("Wu_b"), aggTs[:], start=False, stop=True)
            tauT = pq.tile([128, QT * D], dt.bfloat16, tag="tauT")
            nc.scalar.activation(tauT[:], ps_z, AF.Tanh, bias=hb_upd4, scale=0.5)

            nc.tensor.matmul(ps_h, W("Wg1_t"), cstq, start=True, stop=False)
            nc.tensor.matmul(ps_h, W("Wg1_b"), mnT[:], start=False, stop=True)
            hT = pq.tile([128, QT * D], dt.bfloat16, tag="hT")
            nc.scalar.activation(hT[:], ps_h, AF.Relu, bias=b_g14, scale=1.0)

            # func expert pieces that only need taggT/tauT (issue early)
            d2 = pq.tile([128, QT * D], dt.bfloat16, tag="d2")
            nc.vector.tensor_tensor(out=d2[:], in0=taggT[:], in1=cstq,
                                    op=ALU.subtract)
            f1 = pq.tile([128, QT * D], dt.bfloat16, tag="f1")
            nc.vector.scalar_tensor_tensor(out=f1[:], in0=tauT[:], scalar=1.0,
                                           in1=d2[:], op0=ALU.add, op1=ALU.mult)
            funcT = expcat[:, QT * D:2 * QT * D]
            nc.vector.scalar_tensor_tensor(out=funcT, in0=f1[:], scalar=0.5,
                                           in1=cstq, op0=ALU.mult, op1=ALU.add)

            # CNF: 3 Euler steps in T layout (x kept bf16)
            xcur = cstqt
            for st_i in range(N_STEPS):
                xm = xcur[:]
                nc.tensor.matmul(ps_c, W("Wc_t"), xm, start=True, stop=False)
                nc.tensor.matmul(ps_c, W("Wc_b"), mdT, start=False, stop=True)
                vT = pq.tile([128, QT * D], dt.bfloat16, tag=f"vT{st_i}")
                nc.scalar.activation(vT[:], ps_c, AF.Tanh, bias=b_cnf4, scale=1.0)
                if st_i < N_STEPS - 1:
                    xn = pq.tile([128, QT * D], dt.bfloat16, tag=f"xn{st_i}")
                    xo = xn[:]
                else:
                    xo = expcat[:, 2 * QT * D:3 * QT * D]
                nc.vector.scalar_tensor_tensor(out=xo, in0=vT[:], scalar=DT_STEP,
                                               in1=xm, op0=ALU.mult, op1=ALU.add)
                xcur = xn if st_i < N_STEPS - 1 else None

            # ---- gating: softmax over 3 experts ----
            nc.tensor.matmul(ps_x[0:12, 0:QD], wg2k, hT[:], start=True, stop=True)
            eg = pq.tile([128, QT * D], dt.bfloat16, tag="eg")
            nc.scalar.activation(eg[0:12, :], ps_x[0:12, 0:QD], AF.Exp,
                                 bias=bg2c, scale=1.0)
            nc.tensor.matmul(ps_x[0:12, 0:QD], sden, eg[0:12, :], start=True, stop=True)
            rinv = pq.tile([128, QT * D], dt.float32, tag="rinv")
            nc.vector.reciprocal_approx_fast(rinv[0:12, :], ps_x[0:12, 0:QD])
            gts = pq.tile([128, QT * D], dt.bfloat16, tag="gts")
            nc.vector.tensor_tensor(out=gts[0:12, :], in0=eg[0:12, :],
                                    in1=rinv[0:12, :], op=ALU.mult)

            # replicate gates over feature partitions: [(g,d), (t,c)] x3
            wrep = pq.tile([128, 3 * QT * D], dt.bfloat16, tag="wrep")
            for e in range(3):
                nc.tensor.matmul(ps_we, W(f"REPe{e}")[0:12, :], gts[0:12, :],
                                 start=True, stop=True)
                nc.scalar.copy(wrep[:, e * QT * D:(e + 1) * QT * D], ps_we)

            # ---- weighted combine: one wide gate-multiply, then 2 adds ----
            ac = pq.tile([128, 3 * QT * D], dt.bfloat16, tag="ac")
            nc.vector.tensor_tensor(out=ac[:], in0=expcat[:], in1=wrep[:],
                                    op=ALU.mult)
            s01 = pq.tile([128, QT * D], dt.bfloat16, tag="s01")
            nc.vector.tensor_tensor(out=s01[:], in0=ac[:, 0:QT * D],
                                    in1=ac[:, QT * D:2 * QT * D], op=ALU.add)
            outq = pq.tile([128, QT * D], dt.float32, tag="outq")
            nc.vector.tensor_tensor(out=outq[:], in0=s01[:],
                                    in1=ac[:, 2 * QT * D:3 * QT * D], op=ALU.add)

            nc.sync.dma_start(a_out[:, q * QT * D:(q + 1) * QT * D], outq[:])

        # software pipeline: phase B runs one group behind phase A
        prev = None
        for q in range(nq):
            cur = phase_a(q)
            if prev is not None:
                phase_b(prev)
            prev = cur
        phase_b(prev)


# ---------------------------------------------------------------------------
# host staging
# ---------------------------------------------------------------------------

def _kron4(w):
    return np.kron(np.eye(4, dtype=np.float32), w)


def stage_inputs(inputs, bs=BS, ct=CT):
    nt = bs // ct
    f32 = np.float32
    cs = np.asarray(inputs["current_state"], f32)
    nb = np.asarray(inputs["neighbor_states"], f32)
    tiers = np.asarray(inputs["tier_ids"], np.int32)

    W_local = np.asarray(inputs["W_local"], f32)
    W_msg = np.asarray(inputs["W_msg"], f32)
    W_upd = np.asarray(inputs["W_upd"], f32)
    W_cnf = np.asarray(inputs["W_cnf"], f32)
    W_g1 = np.asarray(inputs["W_g1"], f32)
    W_g2 = np.asarray(inputs["W_g2"], f32)
    b_msg = np.asarray(inputs["b_msg"], f32)
    b_local = np.asarray(inputs["b_local"], f32)
    b_upd = np.asarray(inputs["b_upd"], f32)
    b_cnf = np.asarray(inputs["b_cnf"], f32)
    b_g1 = np.asarray(inputs["b_g1"], f32)
    b_g2 = np.asarray(inputs["b_g2"], f32)

    # --- weight constants ---
    wcq = np.zeros((128, WC_COLS), f32)

    def put(name, m):
        wcq[:m.shape[0], _wslot(name):_wslot(name) + m.shape[1]] = m

    put("W4msg", _kron4(W_msg))
    put("Wl_t", _kron4(W_local[:D]))
    put("Wl_b", _kron4(W_local[D:]))
    put("Wu_t", _kron4(W_upd[:D]))
    put("Wu_b", _kron4(W_upd[D:]))
    put("Wc_t", _kron4(W_cnf[:D]))
    put("Wc_b", _kron4(W_cnf[D:]))
    put("Wg1_t", _kron4(W_g1[:D]))
    put("Wg1_b", _kron4(W_g1[D:] / K))
    put("I128", np.eye(128, dtype=f32))
    put("REP4", np.kron(np.eye(4, dtype=f32), np.ones((1, D), f32)))
    for e in range(3):
        sel = np.zeros((3, D), f32)
        sel[e, :] = 1.0
        put(f"REPe{e}", np.kron(np.eye(4, dtype=f32), sel))
    base = 128 * len(_WSLOTS)
    wcq[:, base:base + 12] = np.kron(np.eye(4, dtype=f32), W_g2)
    wcq[0:12, base + 12:base + 24] = np.kron(np.eye(4, dtype=f32),
                                             np.ones((3, 3), f32))
    wcq = wcq.astype(bf16)

    bcq = np.zeros((128, BC_COLS), f32)
    bcq[:, 0] = np.tile(b_msg, 4)
    bcq[:, 1] = np.tile(b_local, 4)
    bcq[:, 2] = 0.5 * np.tile(b_upd, 4)
    bcq[:, 3] = np.tile(b_cnf, 4)
    bcq[:, 4] = np.tile(b_g1, 4)
    bcq[0:12, 5] = np.tile(b_g2, 4)

    in_maps = []
    for c in range(N_CORES):
        rs = slice(c * bs, (c + 1) * bs)
        nb_c = nb[rs]
        cs_c = cs[rs]
        tr_c = tiers[rs]

        m0 = (tr_c == 0)
        m1 = (tr_c == 1)
        m2 = (tr_c == 2)
        cnt0 = np.maximum(m0.sum(-1), 1).astype(f32)
        cnt1 = np.maximum(m1.sum(-1), 1).astype(f32)
        cnt2 = np.maximum(m2.sum(-1), 1).astype(f32)

        # T layout, tier-1 premasked: nbt[(g,d), t, (c,k)]
        nbm = nb_c * m1[:, :, None].astype(f32)
        arr = nbm.reshape(nt, 4, 32, K, D).transpose(1, 4, 0, 2, 3)
        nbt = np.ascontiguousarray(arr).reshape(128, nt, FR).astype(bf16)

        # natural (d,k): nbn[cell, d*K + k]
        nbn = nb_c.transpose(0, 2, 1).reshape(nt, 128, FR).astype(bf16)

        aux = np.empty((bs, 2 * K), f16)
        aux[:, 0:K] = (m0 / cnt0[:, None]).astype(f16)
        aux[:, K:2 * K] = (m2 / cnt2[:, None]).astype(f16)

        nbig = np.empty((128, nt, TW), np.uint16)
        nbig[:, :, 0:FR] = nbt.view(np.uint16)
        nbig[:, :, FR:2 * FR] = nbn.view(np.uint16).transpose(1, 0, 2)
        nbig[:, :, 2 * FR:TW] = aux.view(np.uint16) \
            .reshape(nt, 128, 2 * K).transpose(1, 0, 2)
        nbig = nbig.reshape(128, nt * TW).view(bf16)

        cst = cs_c.reshape(nt, 4, 32, D).transpose(1, 3, 0, 2) \
            .reshape(128, nt, D).astype(bf16)
        scl = (1.0 / cnt1).reshape(nt, 4, 32).transpose(1, 0, 2) \
            .reshape(4, nt, D).astype(bf16)
        cstm = np.zeros((128, nt, 64), bf16)
        cstm[:, :, 0:D] = cst
        cstm[0:4, :, D:2 * D] = scl
        cstm = cstm.reshape(128, nt * 64)

        in_maps.append({
            "nbig": nbig, "cstm": cstm, "wc": wcq, "bc": bcq,
        })
    return in_maps


def unstage_output(outt, bs=BS, ct=CT):
    """outt [128, nt*D] T layout -> [bs, D] natural."""
    nt = bs // ct
    return np.ascontiguousarray(
        outt.reshape(4, D, nt, 32).transpose(2, 0, 3, 1).reshape(bs, D))


_PROGRAM_CACHE = {}


def kernel(**inputs):
    from concourse.bass_utils import run_bass_kernel_spmd

    key = (BS, CT)
    if key not in _PROGRAM_CACHE:
        _PROGRAM_CACHE[key] = build_program(BS, CT)
    nc = _PROGRAM_CACHE[key]

    in_maps = stage_inputs(inputs, BS, CT)
    res = run_bass_kernel_spmd(nc, in_maps, core_ids=list(range(N_CORES)))
    out = np.concatenate(
        [unstage_output(r["outt"].astype(np.float32)) for r in res.results],
        axis=0)
    return out.astype(np.float32)


# revision 15
# speedup vs baseline: 1.0917x; 1.0141x over previous
"""Trainium2 Bass kernel for nn_MoEConnectionProcessor.

Self-contained: stages/shards the full inputs on host (numpy), runs an SPMD
Bass/Tile kernel on 8 NeuronCores, gathers the full output.

Reference math (per cell, K=26 neighbors, D=32):
  masks by tier (0=local,1=functional,2=distant); masked neighbor means;
  local expert  = tanh([cs, loc_mean] @ W_local + b_local)
  func expert   = (1-z)*cs + z*tanh(agg),  z = sigmoid([cs, agg] @ W_upd + b_upd)
                  agg = masked_mean_k tanh(nb @ W_msg + b_msg)
  dist expert   = 3-step Euler: x += (1/3) tanh([x, agg_d] @ W_cnf + b_cnf)
  gates         = softmax([cs, mean_nb] @ W_g1 + b_g1 -> relu -> @ W_g2 + b_g2)
  out           = sum_t gate_t * expert_t

Device strategy (per 128-cell tile, Q=4 tiles batched for the small ops):
  - nb staged twice from host: T layout [(g,d), (c,k)] PRE-MASKED by the
    tier-1 mask (so tanh gives exact zeros for non-functional neighbors and
    the functional aggregate is a plain k-reduce), and natural [c, (d,k)]
    raw with k innermost (so the masked multiplies run in DVE 2x mode with
    the per-(cell,k) weights broadcast along d as an outer dim).
  - tier-0/tier-2 means: one fused 2x multiply by pre-divided weights
    (m_t/cnt_t, fp16) + one fused 1x k-reduce.
  - S0 (gating mean): PE identity-accumulation into PSUM (26 matmuls).
  - sigmoid via tanh identity, relu/exp/copy on ACT: every activation is
    served by the "exp_and_others" table -> zero ACT table reloads.
  - experts/gating/combine all in T layout on [128, 4*32] batched operands;
    per-cell gates/scales replicated across partitions with tiny PE matmuls;
    output staged in T layout, un-transposed on host.
"""

import numpy as np
import ml_dtypes
from contextlib import ExitStack

import concourse.bass as bass
import concourse.bacc as bacc
import concourse.tile as tile
import concourse.mybir as mybir

B, K, D, NH = 262144, 26, 32, 32
N_CORES = 8
BS = B // N_CORES   # 32768 cells per core
CT = 128            # cells per tile
QT = 8              # tiles per batch-group
N_STEPS = 3
DT_STEP = 1.0 / N_STEPS

dt = mybir.dt
bf16 = ml_dtypes.bfloat16
f16 = np.float16
AF = mybir.ActivationFunctionType
ALU = mybir.AluOpType
AXX = mybir.AxisListType

FR = K * D  # 832
TW = 2 * FR + 2 * K  # 1716: [nbt 832 | nbn 832 | aux 52] packed per tile

# weight-constant dram tensor [128, WC_COLS] bf16 layout
_WSLOTS = ["W4msg", "Wl_t", "Wl_b", "Wu_t", "Wu_b", "Wc_t", "Wc_b",
           "Wg1_t", "Wg1_b", "I128", "REP4", "REPe0", "REPe1", "REPe2"]
_WEXTRA = 24  # WG2K [128,12] + SDEN [12,12 in a 12-col slot]
WC_COLS = 128 * len(_WSLOTS) + _WEXTRA
BC_COLS = 8


def _wslot(name):
    return 128 * _WSLOTS.index(name)


def build_program(bs=BS, ct=CT):
    nt = bs // ct
    nq = nt // QT
    nc = bacc.Bacc("TRN2", target_bir_lowering=False, debug=False,
                   num_devices=N_CORES)

    a_nbig = nc.dram_tensor("nbig", [128, nt * TW], dt.bfloat16, kind="ExternalInput").ap()
    a_cstm = nc.dram_tensor("cstm", [128, nt * 64], dt.bfloat16, kind="ExternalInput").ap()
    a_wc = nc.dram_tensor("wc", [128, WC_COLS], dt.bfloat16, kind="ExternalInput").ap()
    a_bc = nc.dram_tensor("bc", [128, BC_COLS], dt.float32, kind="ExternalInput").ap()
    a_out = nc.dram_tensor("outt", [128, nt * D], dt.float32, kind="ExternalOutput").ap()

    with tile.TileContext(nc) as tc:
        _body(tc, a_nbig, a_cstm, a_wc, a_bc, a_out, bs, ct, nt, nq)
    nc.compile()
    return nc


def _body(tc, a_nbig, a_cstm, a_wc, a_bc, a_out, bs, ct, nt, nq):
    nc = tc.nc

    with ExitStack() as ctx:
        ctx.enter_context(nc.allow_low_precision("reduce output downcast; fp32 internal accum"))
        cpool = ctx.enter_context(tc.tile_pool(name="const", bufs=1))
        pin = ctx.enter_context(tc.tile_pool(name="in", bufs=3))
        pmid = ctx.enter_context(tc.tile_pool(name="mid", bufs=3))
        pq = ctx.enter_context(tc.tile_pool(name="q", bufs=3))
        psm = ctx.enter_context(tc.tile_pool(name="psm", bufs=1, space="PSUM"))
        psq = ctx.enter_context(tc.tile_pool(name="psq", bufs=2, space="PSUM"))
        psb = ctx.enter_context(tc.tile_pool(name="psb", bufs=1, space="PSUM"))

        wc = cpool.tile([128, WC_COLS], dt.bfloat16, tag="wc")
        nc.sync.dma_start(wc[:], a_wc)
        bc = cpool.tile([128, BC_COLS], dt.float32, tag="bc")
        nc.sync.dma_start(bc[:], a_bc)

        def W(name):
            return wc[:, _wslot(name): _wslot(name) + 128]

        wg2k = wc[:, 128 * len(_WSLOTS): 128 * len(_WSLOTS) + 12]
        sden = wc[0:12, 128 * len(_WSLOTS) + 12: 128 * len(_WSLOTS) + 24]
        b_msg4 = bc[:, 0:1]
        b_loc4 = bc[:, 1:2]
        hb_upd4 = bc[:, 2:3]   # 0.5 * b_upd (for the tanh-sigmoid identity)
        b_cnf4 = bc[:, 3:4]
        b_g14 = bc[:, 4:5]
        bg2c = bc[0:12, 5:6]   # b_g2 at (g,e) partitions

        def phase_a(q):
            ML = pq.tile([128, QT * 2 * D], dt.bfloat16, tag="ML")   # [c,(t,s,d)]
            SGT = pq.tile([128, QT * D], dt.bfloat16, tag="SGT")     # [(g,j),(t,c)]
            ps_s0 = psq.tile([128, QT * D], dt.float32, tag="ps_s0")  # [c,(t,d)]
            cm = pin.tile([128, QT * 64], dt.bfloat16, tag="cm")
            nc.sync.dma_start(cm[:], a_cstm[:, q * QT * 64:(q + 1) * QT * 64])

            for s in range(QT):
                t = q * QT + s

                big = pin.tile([128, TW], dt.bfloat16, tag="big")
                nc.sync.dma_start(big[:], a_nbig[:, t * TW:(t + 1) * TW])
                nbt = big[:, 0:FR]
                nbn = big[:, FR:2 * FR]
                aux = big[:, 2 * FR:TW].bitcast(dt.float16)

                nbn3 = nbn.rearrange("p (d k) -> p d k", d=D)

                # tier-0 product on DVE, tier-2 product on GpSimd
                prod = pmid.tile([128, 2 * FR], dt.bfloat16, tag="prod")
                pview = prod[:].rearrange("p (s d k) -> p s d k", s=2, d=D)
                aview = aux.rearrange("p (s k) -> p s k", s=2)
                nc.vector.tensor_tensor(
                    out=pview[:, 0],
                    in0=nbn3,
                    in1=aview[:, 0].unsqueeze(1).to_broadcast((128, D, K)),
                    op=ALU.mult)
                nc.gpsimd.tensor_tensor(
                    out=pview[:, 1],
                    in0=nbn3,
                    in1=aview[:, 1].unsqueeze(1).to_broadcast((128, D, K)),
                    op=ALU.mult)

                # halve k by one 2x pair-add, then 1x-reduce 13 -> means
                padd = pmid.tile([128, 2 * D * 13], dt.bfloat16, tag="padd")
                pav = padd[:].rearrange("p (s d k) -> p s d k", s=2, d=D)
                nc.vector.tensor_tensor(out=pav, in0=pview[:, :, :, 0:13],
                                        in1=pview[:, :, :, 13:26], op=ALU.add)
                mlv = ML[:].rearrange("p (t s d) -> p t s d", t=QT, s=2)
                nc.vector.tensor_reduce(out=mlv[:, s], in_=pav,
                                        axis=AXX.X, op=ALU.add)

                # messages: premsgs = kron4(W_msg) @ nbt  (nbt pre-masked m1)
                ps_m0 = psm.tile([128, 416], dt.float32, tag="psm0")
                ps_m1 = psm.tile([128, 416], dt.float32, tag="psm1")
                nc.tensor.matmul(ps_m0[:], W("W4msg"), nbt[:, 0:416],
                                 start=True, stop=True)
                nc.tensor.matmul(ps_m1[:], W("W4msg"), nbt[:, 416:832],
                                 start=True, stop=True)
                mt = pmid.tile([128, FR], dt.bfloat16, tag="msgsT")
                nc.scalar.activation(mt[:, 0:416], ps_m0[:], AF.Tanh,
                                     bias=b_msg4, scale=1.0)
                nc.scalar.activation(mt[:, 416:832], ps_m1[:], AF.Tanh,
                                     bias=b_msg4, scale=1.0)

                # functional aggregate: 2x pair-add + 1x k-reduce
                mt3 = mt[:].rearrange("p (c k) -> p c k", k=K)
                spad = pmid.tile([128, D * 13], dt.bfloat16, tag="spad")
                spv = spad[:].rearrange("p (c k) -> p c k", k=13)
                nc.vector.tensor_tensor(out=spv, in0=mt3[:, :, 0:13],
                                        in1=mt3[:, :, 13:26], op=ALU.add)
                nc.vector.tensor_reduce(out=SGT[:, s * D:(s + 1) * D],
                                        in_=spv, axis=AXX.X, op=ALU.add)

                # S0 = sum_k nb  (PE identity accumulation)
                for k in range(K):
                    nc.tensor.matmul(ps_s0[:, s * D:(s + 1) * D], W("I128"),
                                     nbn3[:, :, k],
                                     start=(k == 0), stop=(k == K - 1))

            return dict(ML=ML, SGT=SGT, ps_s0=ps_s0, cm=cm, q=q)

        def phase_b(st):
            ML, SGT, ps_s0, cm, q = st["ML"], st["SGT"], st["ps_s0"], st["cm"], st["q"]

            cmv = cm[:].rearrange("p (t x) -> p t x", x=64)
            cstqt = pq.tile([128, QT * D], dt.bfloat16, tag="cstqt")
            nc.vector.tensor_copy(cstqt[:], cmv[:, :, 0:D])
            cstq = cstqt[:]
            sclq4 = cm[0:4, :].rearrange("p (t x) -> p t x", x=64)[:, :, D:2 * D]

            # T-transpose of the tier means (32x32 block transpose)
            MLT = pq.tile([128, QT * 2 * D], dt.bfloat16, tag="MLT")
            nc.vector.transpose(MLT[:], ML[:])
            mltv = MLT[:].rearrange("p (t s c) -> p t s c", t=QT, s=2)
            mlT = mltv[:, :, 0]   # [(g,d), (t,c)] tier-0 means
            mdT = mltv[:, :, 1]   # tier-2 means

            # S0 -> bf16 -> T layout (gating mean; 1/K folded into Wg1_b)
            s0b = pq.tile([128, QT * D], dt.bfloat16, tag="s0b")
            nc.scalar.copy(s0b[:], ps_s0[:])
            mnT = pq.tile([128, QT * D], dt.bfloat16, tag="mnT")
            nc.vector.transpose(mnT[:], s0b[:])

            # shared PSUM bank tiles for all phase-B matmul outputs
            QD = QT * D
            PB1a = psb.tile([128, 2 * QD], dt.float32, tag="PB1a")
            PB1b = psb.tile([128, 2 * QD], dt.float32, tag="PB1b")
            PB2 = psb.tile([128, 2 * QD], dt.float32, tag="PB2")
            ps_l = PB1a[:, 0:QD]
            ps_z = PB1a[:, QD:2 * QD]
            ps_h = PB1b[:, 0:QD]
            ps_c = PB1b[:, QD:2 * QD]
            ps_x = PB2[:, 0:QD]      # reused: scl-rep, gating logits, denom
            ps_we = PB2[:, QD:2 * QD]

            # replicate 1/cnt1 across partitions; scale + tanh the aggregate
            nc.tensor.matmul(ps_x, W("REP4")[0:4, :], sclq4,
                             start=True, stop=True)
            aggTs = pq.tile([128, QT * D], dt.bfloat16, tag="aggTs")
            nc.vector.tensor_tensor(out=aggTs[:], in0=SGT[:], in1=ps_x,
                                    op=ALU.mult)
            taggT = pq.tile([128, QT * D], dt.bfloat16, tag="taggT")
            nc.scalar.activation(taggT[:], aggTs[:], AF.Tanh)

            # ---- experts (batched matmuls over [128, QT*32]) ----
            nc.tensor.matmul(ps_l, W("Wl_t"), cstq, start=True, stop=False)
            nc.tensor.matmul(ps_l, W("Wl_b"), mlT, start=False, stop=True)
            expcat = pq.tile([128, 3 * QT * D], dt.bfloat16, tag="expcat")
            localT = expcat[:, 0:QT * D]
            nc.scalar.activation(localT, ps_l, AF.Tanh, bias=b_loc4, scale=1.0)

            nc.tensor.matmul(ps_z, W("Wu_t"), cstq, start=True, stop=False)
            nc.tensor.matmul(ps_z, W("Wu_b"), aggTs[:], start=False, stop=True)
            tauT = pq.tile([128, QT * D], dt.bfloat16, tag="tauT")
            nc.scalar.activation(tauT[:], ps_z, AF.Tanh, bias=hb_upd4, scale=0.5)

            nc.tensor.matmul(ps_h, W("Wg1_t"), cstq, start=True, stop=False)
            nc.tensor.matmul(ps_h, W("Wg1_b"), mnT[:], start=False, stop=True)
            hT = pq.tile([128, QT * D], dt.bfloat16, tag="hT")
            nc.scalar.activation(hT[:], ps_h, AF.Relu, bias=b_g14, scale=1.0)

            # CNF: 3 Euler steps in T layout (x kept bf16)
            xcur = cstqt
            for st_i in range(N_STEPS):
                xm = xcur[:]
                nc.tensor.matmul(ps_c, W("Wc_t"), xm, start=True, stop=False)
                nc.tensor.matmul(ps_c, W("Wc_b"), mdT, start=False, stop=True)
                vT = pq.tile([128, QT * D], dt.bfloat16, tag=f"vT{st_i}")
                nc.scalar.activation(vT[:], ps_c, AF.Tanh, bias=b_cnf4, scale=1.0)
                if st_i < N_STEPS - 1:
                    xn = pq.tile([128, QT * D], dt.bfloat16, tag=f"xn{st_i}")
                    xo = xn[:]
                else:
                    xo = expcat[:, 2 * QT * D:3 * QT * D]
                nc.vector.scalar_tensor_tensor(out=xo, in0=vT[:], scalar=DT_STEP,
                                               in1=xm, op0=ALU.mult, op1=ALU.add)
                xcur = xn if st_i < N_STEPS - 1 else None

            # func expert: cs + (0.5 + 0.5*tau) * (tanh(agg) - cs)
            d2 = pq.tile([128, QT * D], dt.bfloat16, tag="d2")
            nc.vector.tensor_tensor(out=d2[:], in0=taggT[:], in1=cstq,
                                    op=ALU.subtract)
            f1 = pq.tile([128, QT * D], dt.bfloat16, tag="f1")
            nc.vector.scalar_tensor_tensor(out=f1[:], in0=tauT[:], scalar=1.0,
                                           in1=d2[:], op0=ALU.add, op1=ALU.mult)
            funcT = expcat[:, QT * D:2 * QT * D]
            nc.vector.scalar_tensor_tensor(out=funcT, in0=f1[:], scalar=0.5,
                                           in1=cstq, op0=ALU.mult, op1=ALU.add)

            # ---- gating: softmax over 3 experts ----
            nc.tensor.matmul(ps_x[0:12, 0:QD], wg2k, hT[:], start=True, stop=True)
            eg = pq.tile([128, QT * D], dt.bfloat16, tag="eg")
            nc.scalar.activation(eg[0:12, :], ps_x[0:12, 0:QD], AF.Exp,
                                 bias=bg2c, scale=1.0)
            nc.tensor.matmul(ps_x[0:12, 0:QD], sden, eg[0:12, :], start=True, stop=True)
            rinv = pq.tile([128, QT * D], dt.float32, tag="rinv")
            nc.vector.reciprocal_approx_fast(rinv[0:12, :], ps_x[0:12, 0:QD])
            gts = pq.tile([128, QT * D], dt.bfloat16, tag="gts")
            nc.vector.tensor_tensor(out=gts[0:12, :], in0=eg[0:12, :],
                                    in1=rinv[0:12, :], op=ALU.mult)

            # replicate gates over feature partitions: [(g,d), (t,c)] x3
            wrep = pq.tile([128, 3 * QT * D], dt.bfloat16, tag="wrep")
            for e in range(3):
                nc.tensor.matmul(ps_we, W(f"REPe{e}")[0:12, :], gts[0:12, :],
                                 start=True, stop=True)
                nc.scalar.copy(wrep[:, e * QT * D:(e + 1) * QT * D], ps_we)

            # ---- weighted combine: one wide gate-multiply, then 2 adds ----
            ac = pq.tile([128, 3 * QT * D], dt.bfloat16, tag="ac")
            nc.vector.tensor_tensor(out=ac[:], in0=expcat[:], in1=wrep[:],
                                    op=ALU.mult)
            s01 = pq.tile([128, QT * D], dt.bfloat16, tag="s01")
            nc.vector.tensor_tensor(out=s01[:], in0=ac[:, 0:QT * D],
                                    in1=ac[:, QT * D:2 * QT * D], op=ALU.add)
            outq = pq.tile([128, QT * D], dt.float32, tag="outq")
            nc.vector.tensor_tensor(out=outq[:], in0=s01[:],
                                    in1=ac[:, 2 * QT * D:3 * QT * D], op=ALU.add)

            nc.sync.dma_start(a_out[:, q * QT * D:(q + 1) * QT * D], outq[:])

        # software pipeline: phase B runs one group behind phase A
        prev = None
        for q in range(nq):
            cur = phase_a(q)
            if prev is not None:
                phase_b(prev)
            prev = cur
        phase_b(prev)


# ---------------------------------------------------------------------------
# host staging
# ---------------------------------------------------------------------------

def _kron4(w):
    return np.kron(np.eye(4, dtype=np.float32), w)


def stage_inputs(inputs, bs=BS, ct=CT):
    nt = bs // ct
    f32 = np.float32
    cs = np.asarray(inputs["current_state"], f32)
    nb = np.asarray(inputs["neighbor_states"], f32)
    tiers = np.asarray(inputs["tier_ids"], np.int32)

    W_local = np.asarray(inputs["W_local"], f32)
    W_msg = np.asarray(inputs["W_msg"], f32)
    W_upd = np.asarray(inputs["W_upd"], f32)
    W_cnf = np.asarray(inputs["W_cnf"], f32)
    W_g1 = np.asarray(inputs["W_g1"], f32)
    W_g2 = np.asarray(inputs["W_g2"], f32)
    b_msg = np.asarray(inputs["b_msg"], f32)
    b_local = np.asarray(inputs["b_local"], f32)
    b_upd = np.asarray(inputs["b_upd"], f32)
    b_cnf = np.asarray(inputs["b_cnf"], f32)
    b_g1 = np.asarray(inputs["b_g1"], f32)
    b_g2 = np.asarray(inputs["b_g2"], f32)

    # --- weight constants ---
    wcq = np.zeros((128, WC_COLS), f32)

    def put(name, m):
        wcq[:m.shape[0], _wslot(name):_wslot(name) + m.shape[1]] = m

    put("W4msg", _kron4(W_msg))
    put("Wl_t", _kron4(W_local[:D]))
    put("Wl_b", _kron4(W_local[D:]))
    put("Wu_t", _kron4(W_upd[:D]))
    put("Wu_b", _kron4(W_upd[D:]))
    put("Wc_t", _kron4(W_cnf[:D]))
    put("Wc_b", _kron4(W_cnf[D:]))
    put("Wg1_t", _kron4(W_g1[:D]))
    put("Wg1_b", _kron4(W_g1[D:] / K))
    put("I128", np.eye(128, dtype=f32))
    put("REP4", np.kron(np.eye(4, dtype=f32), np.ones((1, D), f32)))
    for e in range(3):
        sel = np.zeros((3, D), f32)
        sel[e, :] = 1.0
        put(f"REPe{e}", np.kron(np.eye(4, dtype=f32), sel))
    base = 128 * len(_WSLOTS)
    wcq[:, base:base + 12] = np.kron(np.eye(4, dtype=f32), W_g2)
    wcq[0:12, base + 12:base + 24] = np.kron(np.eye(4, dtype=f32),
                                             np.ones((3, 3), f32))
    wcq = wcq.astype(bf16)

    bcq = np.zeros((128, BC_COLS), f32)
    bcq[:, 0] = np.tile(b_msg, 4)
    bcq[:, 1] = np.tile(b_local, 4)
    bcq[:, 2] = 0.5 * np.tile(b_upd, 4)
    bcq[:, 3] = np.tile(b_cnf, 4)
    bcq[:, 4] = np.tile(b_g1, 4)
    bcq[0:12, 5] = np.tile(b_g2, 4)

    in_maps = []
    for c in range(N_CORES):
        rs = slice(c * bs, (c + 1) * bs)
        nb_c = nb[rs]
        cs_c = cs[rs]
        tr_c = tiers[rs]

        m0 = (tr_c == 0)
        m1 = (tr_c == 1)
        m2 = (tr_c == 2)
        cnt0 = np.maximum(m0.sum(-1), 1).astype(f32)
        cnt1 = np.maximum(m1.sum(-1), 1).astype(f32)
        cnt2 = np.maximum(m2.sum(-1), 1).astype(f32)

        # T layout, tier-1 premasked: nbt[(g,d), t, (c,k)]
        nbm = nb_c * m1[:, :, None].astype(f32)
        arr = nbm.reshape(nt, 4, 32, K, D).transpose(1, 4, 0, 2, 3)
        nbt = np.ascontiguousarray(arr).reshape(128, nt, FR).astype(bf16)

        # natural (d,k): nbn[cell, d*K + k]
        nbn = nb_c.transpose(0, 2, 1).reshape(nt, 128, FR).astype(bf16)

        aux = np.empty((bs, 2 * K), f16)
        aux[:, 0:K] = (m0 / cnt0[:, None]).astype(f16)
        aux[:, K:2 * K] = (m2 / cnt2[:, None]).astype(f16)

        nbig = np.empty((128, nt, TW), np.uint16)
        nbig[:, :, 0:FR] = nbt.view(np.uint16)
        nbig[:, :, FR:2 * FR] = nbn.view(np.uint16).transpose(1, 0, 2)
        nbig[:, :, 2 * FR:TW] = aux.view(np.uint16) \
            .reshape(nt, 128, 2 * K).transpose(1, 0, 2)
        nbig = nbig.reshape(128, nt * TW).view(bf16)

        cst = cs_c.reshape(nt, 4, 32, D).transpose(1, 3, 0, 2) \
            .reshape(128, nt, D).astype(bf16)
        scl = (1.0 / cnt1).reshape(nt, 4, 32).transpose(1, 0, 2) \
            .reshape(4, nt, D).astype(bf16)
        cstm = np.zeros((128, nt, 64), bf16)
        cstm[:, :, 0:D] = cst
        cstm[0:4, :, D:2 * D] = scl
        cstm = cstm.reshape(128, nt * 64)

        in_maps.append({
            "nbig": nbig, "cstm": cstm, "wc": wcq, "bc": bcq,
        })
    return in_maps


def unstage_output(outt, bs=BS, ct=CT):
    """outt [128, nt*D] T layout -> [bs, D] natural."""
    nt = bs // ct
    return np.ascontiguousarray(
        outt.reshape(4, D, nt, 32).transpose(2, 0, 3, 1).reshape(bs, D))


_PROGRAM_CACHE = {}


def kernel(**inputs):
    from concourse.bass_utils import run_bass_kernel_spmd

    key = (BS, CT)
    if key not in _PROGRAM_CACHE:
        _PROGRAM_CACHE[key] = build_program(BS, CT)
    nc = _PROGRAM_CACHE[key]

    in_maps = stage_inputs(inputs, BS, CT)
    res = run_bass_kernel_spmd(nc, in_maps, core_ids=list(range(N_CORES)))
    out = np.concatenate(
        [unstage_output(r["outt"].astype(np.float32)) for r in res.results],
        axis=0)
    return out.astype(np.float32)
